# revision 1
# baseline (speedup 1.0000x reference)
"""Memristor linear layer kernel for 8 TRN2 NeuronCores.

The reference memristor crossbar computation collapses algebraically to
    out = x @ weights.T + bias
(the G_OFF offsets cancel in the pos/neg column subtraction and the k_G /
k_I scale factors cancel exactly), so the kernel computes the plain linear
layer.

Precision: fp32 operands are split on host into bf16 hi + bf16 lo halves;
the device computes hi*hi + hi*lo + lo*hi with fp32 PSUM accumulation
(~4e-6 relative error vs 3e-7 for native fp32) at full bf16 PE rate.

Sharding: tensor-parallel over the 1024 output features -> 128 per core.
Each core receives x.T (replicated) and its W.T column shard, pre-packed
on host into the exact SBUF layout [128 partitions, k_tile, free] so
every DMA moves per-partition-contiguous rows at line rate. Each core
computes its out.T shard [128, 256] = W_shard @ x.T + bias accumulated
over 8 K-chunks of 128 in PSUM. Host concatenates and transposes back.

Schedule notes (from NTFF profiling on TRN2 under axon):
- The HWDGE rings drain in global issue order at ~280 GB/s, with ~1 us
  per-transfer completion latency, so transfers are staged in the exact
  order the matmul passes need them (wh | xh halves, wl, xl halves).
- The PE HAM clock gate needs ~3.4 us of sustained busy-ness to release
  (1.2 -> 2.4 GHz) and re-throttles after ~2 us of idle, so garbage
  warm-up matmuls run while DMAs stream and tiny filler matmuls are
  interleaved between compute passes to bridge DMA chase-stalls.
"""

import os

import numpy as np

BATCH = 256
SIZE_IN = 1024
SIZE_OUT = 1024
N_CORES = 8
O_SHARD = SIZE_OUT // N_CORES  # 128
K_TILES = SIZE_IN // 128  # 8

_STATE = {}


def _build():
    import concourse.bass as bass
    import concourse.tile as tile
    from concourse import bacc, mybir

    f32 = mybir.dt.float32
    bf16 = mybir.dt.bfloat16
    n_warm = int(os.environ.get("WARMUP_MM", "5"))

    nc = bacc.Bacc(None, target_bir_lowering=False)

    # All tensors pre-packed on host to [128, ..., free] (partition major)
    # so every DMA descriptor is a large per-partition-contiguous run.
    xh_d = nc.declare_dram_parameter("xh", [128, K_TILES, BATCH], bf16, isOutput=False)
    xl_d = nc.declare_dram_parameter("xl", [128, K_TILES, BATCH], bf16, isOutput=False)
    whl_d = nc.declare_dram_parameter(
        "whl", [128, 2, K_TILES, O_SHARD], bf16, isOutput=False
    )
    b_d = nc.declare_dram_parameter("bias", [O_SHARD, 1], f32, isOutput=False)
    out_d = nc.declare_dram_parameter("out", [O_SHARD, BATCH], f32, isOutput=True)

    with tile.TileContext(nc) as tc:
        with (
            tc.tile_pool(name="sbuf", bufs=1) as pool,
            tc.tile_pool(name="psum", bufs=1, space="PSUM") as psum_pool,
        ):
            xh_s = pool.tile([128, K_TILES, BATCH], bf16)
            xl_s = pool.tile([128, K_TILES, BATCH], bf16)
            whl_s = pool.tile([128, 2, K_TILES, O_SHARD], bf16)
            b_s = pool.tile([O_SHARD, 1], f32)
            o_s = pool.tile([O_SHARD, BATCH], f32)
            pt = psum_pool.tile([O_SHARD, BATCH], f32)

            # PE warm-up: garbage matmuls into a scratch PSUM bank so the
            # HAM clock-gate releases (1.2 -> 2.4 GHz) while DMAs stream.
            # A few big ones build the busy window, then small (~54 ns)
            # ones keep PE occupied at fine granularity until real data
            # lands; more small ones are interleaved between the compute
            # passes below so DMA chase-stalls can't re-throttle the PE.
            n_warm_small = int(os.environ.get("WARMUP_MM_SMALL", "30"))
            warm_in = pool.tile([128, 512], bf16)
            warm_ps = psum_pool.tile([128, 512], f32)
            nc.vector.memset(warm_in[:], 0.0)

            def warm_big(n):
                for _ in range(n):
                    nc.tensor.matmul(
                        warm_ps[:], warm_in[:, 0:128], warm_in[:], start=True,
                        stop=True,
                    )

            def warm_small(n):
                for _ in range(n):
                    nc.tensor.matmul(
                        warm_ps[:, 0:64], warm_in[:, 0:128], warm_in[:, 0:64],
                        start=True, stop=True,
                    )

            warm_big(n_warm)
            warm_small(n_warm_small)

            # Fine-grained transfers. Each engine issues its own queue in
            # program order and the HWDGE drains in global issue-time
            # order, so keep everything whose order matters on the scalar
            # ring; sync carries only the two wh halves issued up front.
            h = K_TILES // 2
            variant = os.environ.get("DMA_VARIANT", "min4")
            if variant == "minw":
                # weights hi+lo and x hi combined in ONE 1MB transfer
                # (8 KB/partition descriptors, one less transfer boundary)
                wx_d = nc.declare_dram_parameter(
                    "wx", [128, 4096], bf16, isOutput=False
                )
                wx_s = pool.tile([128, 4096], bf16)
                nc.sync.dma_start(out=wx_s[:], in_=wx_d[:])
                nc.scalar.dma_start(out=xl_s[:, 0:h, :], in_=xl_d[:, 0:h, :])
                nc.scalar.dma_start(out=xl_s[:, h:, :], in_=xl_d[:, h:, :])

                def wh_k(k):
                    return wx_s[:, k * 128 : (k + 1) * 128]

                def wl_k(k):
                    return wx_s[:, 1024 + k * 128 : 1024 + (k + 1) * 128]

                def xh_k(k):
                    return wx_s[:, 2048 + k * 256 : 2048 + (k + 1) * 256]

                ap_plan = []
                for k in range(K_TILES):
                    ap_plan.append((wh_k(k), xh_k(k)))
                    ap_plan.append((wl_k(k), xh_k(k)))
                    if k == h - 1:
                        ap_plan.append(None)
                ap_plan.append(None)
                ap_plan += [
                    (wh_k(k), xl_s[:, k, :]) for k in range(K_TILES)
                ]
                plan = None
            elif variant in ("min4", "min4b", "min3"):
                # Minimal transfer count: the kernel end is stream-bound,
                # so per-transfer overhead matters more than fine gating
                # (the PE has slack to absorb coarser chunks).
                nc.sync.dma_start(out=whl_s[:], in_=whl_d[:])
                nc.scalar.dma_start(out=xh_s[:], in_=xh_d[:])
                if variant == "min4":
                    nc.scalar.dma_start(out=xl_s[:, 0:h, :], in_=xl_d[:, 0:h, :])
                    nc.scalar.dma_start(out=xl_s[:, h:, :], in_=xl_d[:, h:, :])
                elif variant == "min4b":
                    # uneven split: tiny last transfer so only 2 matmuls
                    # remain after the stream ends
                    nc.scalar.dma_start(out=xl_s[:, 0:6, :], in_=xl_d[:, 0:6, :])
                    nc.scalar.dma_start(out=xl_s[:, 6:, :], in_=xl_d[:, 6:, :])
                else:
                    nc.scalar.dma_start(out=xl_s[:], in_=xl_d[:])
                plan = []
                for k in range(K_TILES):
                    plan.append((0, xh_s, k))
                    plan.append((1, xh_s, k))
                    if k == h - 1:
                        plan.append(None)
                plan.append(None)
                plan += [(0, xl_s, k) for k in range(K_TILES)]
            elif variant == "par":
                # balanced rings: sync 768KB, scalar 768KB — tests whether
                # the two HWDGE rings can drain concurrently
                nc.sync.dma_start(out=whl_s[:], in_=whl_d[:])
                nc.scalar.dma_start(out=xh_s[:, 0:h, :], in_=xh_d[:, 0:h, :])
                nc.scalar.dma_start(out=xh_s[:, h:, :], in_=xh_d[:, h:, :])
                nc.sync.dma_start(out=xl_s[:, 0:h, :], in_=xl_d[:, 0:h, :])
                nc.scalar.dma_start(out=xl_s[:, h:, :], in_=xl_d[:, h:, :])
                plan = (
                    [(0, xh_s, k) for k in range(h)]
                    + [(1, xh_s, k) for k in range(h)]
                    + [None]
                    + [(0, xh_s, k) for k in range(h, K_TILES)]
                    + [(1, xh_s, k) for k in range(h, K_TILES)]
                    + [None]
                    + [(0, xl_s, k) for k in range(K_TILES)]
                )
            elif variant == "whl1":
                # One early 512 KB weight transfer (hi+lo), then x hi and
                # x lo halves chase on the scalar ring. Both weight halves
                # are ready when the first x chunk lands, so the lo*hi
                # pass interleaves early and only hi*lo waits for x lo.
                nc.sync.dma_start(out=whl_s[:], in_=whl_d[:])
                nc.scalar.dma_start(out=xh_s[:, 0:h, :], in_=xh_d[:, 0:h, :])
                nc.scalar.dma_start(out=xh_s[:, h:, :], in_=xh_d[:, h:, :])
                nc.scalar.dma_start(out=xl_s[:, 0:h, :], in_=xl_d[:, 0:h, :])
                nc.scalar.dma_start(out=xl_s[:, h:, :], in_=xl_d[:, h:, :])
                plan = (
                    [(0, xh_s, k) for k in range(h)]
                    + [(1, xh_s, k) for k in range(h)]
                    + [None]
                    + [(0, xh_s, k) for k in range(h, K_TILES)]
                    + [(1, xh_s, k) for k in range(h, K_TILES)]
                    + [None]
                    + [(0, xl_s, k) for k in range(K_TILES)]
                )
            else:
                # wh | xh halves | wl | xl halves in need order
                nc.sync.dma_start(out=whl_s[:, 0, :, :], in_=whl_d[:, 0, :, :])
                nc.scalar.dma_start(out=xh_s[:, 0:h, :], in_=xh_d[:, 0:h, :])
                nc.scalar.dma_start(out=xh_s[:, h:, :], in_=xh_d[:, h:, :])
                nc.sync.dma_start(out=whl_s[:, 1, :, :], in_=whl_d[:, 1, :, :])
                nc.scalar.dma_start(out=xl_s[:, 0:h, :], in_=xl_d[:, 0:h, :])
                nc.scalar.dma_start(out=xl_s[:, h:, :], in_=xl_d[:, h:, :])
                plan = (
                    [(0, xh_s, k) for k in range(h)]
                    + [None]
                    + [(0, xh_s, k) for k in range(h, K_TILES)]
                    + [None]
                    + [(1, xh_s, k) for k in range(K_TILES)]
                    + [None]
                    + [(0, xl_s, k) for k in range(K_TILES)]
                )
            # bias: tiny transfer; by default on the scalar ring tail so
            # the gpsimd engine (slow SWDGE drain) stays completely idle
            if os.environ.get("BIAS_GPSIMD", "0") == "1":
                nc.gpsimd.dma_start(out=b_s[:], in_=b_d[:])
            else:
                nc.scalar.dma_start(out=b_s[:], in_=b_d[:])
            if plan is not None:
                ap_plan = [
                    (whl_s[:, p[0], p[2], :], p[1][:, p[2], :])
                    if p is not None
                    else None
                    for p in plan
                ]
            n_mm = len([p for p in ap_plan if p is not None])
            i = 0
            for p in ap_plan:
                if p is None:
                    warm_small(int(os.environ.get("WARMUP_MM_GAP", "8")))
                    continue
                nc.tensor.matmul(
                    pt[:],
                    p[0],
                    p[1],
                    start=(i == 0),
                    stop=(i == n_mm - 1),
                )
                i += 1

            # bias-add/copy in halves: the first out-half DMA issues while
            # the second half is still copying; halves ride both HWDGE
            # rings so the completion receipts (~1 us each to HBM) overlap
            hb = BATCH // 2
            if os.environ.get("TS_SPLIT", "1") == "1":
                nc.vector.tensor_scalar_add(
                    out=o_s[:, 0:hb], in0=pt[:, 0:hb], scalar1=b_s[:]
                )
                nc.sync.dma_start(out=out_d[:, 0:hb], in_=o_s[:, 0:hb])
                nc.vector.tensor_scalar_add(
                    out=o_s[:, hb:], in0=pt[:, hb:], scalar1=b_s[:]
                )
                nc.scalar.dma_start(out=out_d[:, hb:], in_=o_s[:, hb:])
            else:
                nc.vector.tensor_scalar_add(out=o_s[:], in0=pt[:], scalar1=b_s[:])
                nc.sync.dma_start(out=out_d[:, 0:hb], in_=o_s[:, 0:hb])
                nc.scalar.dma_start(out=out_d[:, hb:], in_=o_s[:, hb:])

    nc.compile()
    return nc


def _install_ntff_hook_shim():
    """The agent image's antenv lacks axon_hooks; recreate it so
    run_bass_kernel_spmd(trace=True) can capture NTFF profiles."""
    import sys
    import types

    if "antenv.axon_hooks" in sys.modules:
        return
    try:
        import antenv.axon_hooks  # noqa: F401  (real module exists)

        return
    except ImportError:
        pass
    mod = types.ModuleType("antenv.axon_hooks")
    mod._HOOK = None

    def set_axon_ntff_profile_hook(hook):
        mod._HOOK = hook

    def get_axon_ntff_profile_hook():
        return mod._HOOK

    mod.set_axon_ntff_profile_hook = set_axon_ntff_profile_hook
    mod.get_axon_ntff_profile_hook = get_axon_ntff_profile_hook
    sys.modules["antenv.axon_hooks"] = mod
    try:
        from trn_agent_boot.trn_boot import _ntff_profile_via_ctypes

        mod._HOOK = _ntff_profile_via_ctypes("/opt/axon/libaxon_pjrt.so")
    except Exception:
        pass


def _split_pack(a_t: np.ndarray, ncols: int):
    """[SIZE_IN, ncols] f32 -> two bf16 arrays packed as [128, K_TILES, ncols]."""
    import ml_dtypes

    hi = a_t.astype(ml_dtypes.bfloat16)
    lo = (a_t - hi.astype(np.float32)).astype(ml_dtypes.bfloat16)

    def pack(v):
        return np.ascontiguousarray(
            v.reshape(K_TILES, 128, ncols).transpose(1, 0, 2)
        )

    return pack(hi), pack(lo)


def _split_pack_w(w_t: np.ndarray):
    """[SIZE_IN, O_SHARD] f32 -> one bf16 array [128, 2, K_TILES, O_SHARD]
    holding the hi and lo halves contiguously per partition."""
    hi, lo = _split_pack(w_t, O_SHARD)
    return np.ascontiguousarray(np.stack([hi, lo], axis=1))


def kernel(x: np.ndarray, weights: np.ndarray, bias: np.ndarray) -> np.ndarray:
    from concourse.bass_utils import run_bass_kernel_spmd

    if "nc" not in _STATE:
        _STATE["nc"] = _build()
    nc = _STATE["nc"]

    x = np.asarray(x, dtype=np.float32)
    weights = np.asarray(weights, dtype=np.float32)
    bias = np.asarray(bias, dtype=np.float32)

    xt = np.ascontiguousarray(x.T)  # [SIZE_IN, BATCH] f32
    xh, xl = _split_pack(xt, BATCH)
    wt = np.ascontiguousarray(weights.T)  # [SIZE_IN, SIZE_OUT] f32

    minw = os.environ.get("DMA_VARIANT", "min4") == "minw"
    in_maps = []
    for c in range(N_CORES):
        sl = slice(c * O_SHARD, (c + 1) * O_SHARD)
        whl = _split_pack_w(np.ascontiguousarray(wt[:, sl]))
        m = {
            "xh": xh,
            "xl": xl,
            "whl": whl,
            "bias": np.ascontiguousarray(bias[sl]).reshape(O_SHARD, 1),
        }
        if minw:
            m["wx"] = np.ascontiguousarray(
                np.concatenate(
                    [whl.reshape(128, -1), xh.reshape(128, -1)], axis=1
                )
            )
        in_maps.append(m)

    # Always install the shim: if BASS_TRACE is set in the environment,
    # run_bass_kernel_spmd imports antenv.axon_hooks unconditionally and
    # would otherwise crash on images whose antenv lacks that module.
    _install_ntff_hook_shim()
    trace = os.environ.get("BASS_PROBLEM_TRACE", "0") == "1"
    res = run_bass_kernel_spmd(
        nc, in_maps, core_ids=list(range(N_CORES)), trace=trace
    )
    _STATE["last_results"] = res

    out_t = np.concatenate(
        [np.asarray(res.results[c]["out"]) for c in range(N_CORES)], axis=0
    )  # [SIZE_OUT, BATCH]
    return np.ascontiguousarray(out_t.T).astype(np.float32, copy=False)



# revision 2
# speedup vs baseline: 1.3411x; 1.3411x over previous
"""Memristor linear layer kernel for 8 TRN2 NeuronCores.

The reference memristor crossbar computation collapses algebraically to
    out = x @ weights.T + bias
(the G_OFF offsets cancel in the pos/neg column subtraction and the k_G /
k_I scale factors cancel exactly), so the kernel computes the plain linear
layer.

Precision: harness tolerance is rel_err < 2e-2; plain bf16 operands with
fp32 PSUM accumulation and a bf16 output land at ~2.9e-3, so everything
is bf16 (half the HBM traffic of the previous hi+lo split scheme).  The
(always zero per the problem spec) bias is added on host in fp32.

Sharding: tensor-parallel over the 1024 output features -> 128 per core.
Each core receives x.T (replicated, bf16) and its W.T column shard (bf16),
pre-packed on host into the exact SBUF layout [128 partitions, k_tile,
free] so every DMA moves per-partition-contiguous rows at line rate.

Schedule: raw bass (no TileContext) so the first DMA descriptor-gen
instructions sit at the very head of each engine's stream (no tile-entry
all-engine barrier behind the framework const memsets), and the epilogue
is just [wait out-DMA, RANGE_CLEAR] (no tile-exit barrier rounds).
Input streams ride both HWDGE rings concurrently: SP carries w then the
last x chunk; ACT carries the first two x chunks.  PE runs garbage
warm-up matmuls from the stream head so the HAM clock boost (1.2 ->
2.4 GHz after ~3.4 us sustained busy) engages before real data lands.
"""

import os

import numpy as np

BATCH = 256
SIZE_IN = 1024
SIZE_OUT = 1024
N_CORES = 8
O_SHARD = SIZE_OUT // N_CORES  # 128
K_TILES = SIZE_IN // 128  # 8

_STATE = {}


def _build():
    import contextlib

    import concourse.bass as bass  # noqa: F401
    from concourse import bacc, mybir

    f32 = mybir.dt.float32
    bf16 = mybir.dt.bfloat16

    n_warm_big = int(os.environ.get("WARM_BIG", "3"))
    n_warm_small = int(os.environ.get("WARM_SMALL", "20"))
    # x chunk split points (k-tile indices): ACT carries [0,s1) and [s1,s2),
    # SP carries w then [s2,K_TILES)
    s1 = int(os.environ.get("XSPLIT1", "3"))
    s2 = int(os.environ.get("XSPLIT2", "6"))

    nc = bacc.Bacc(None, target_bir_lowering=False)

    w_d = nc.declare_dram_parameter("w", [128, K_TILES, O_SHARD], bf16, isOutput=False)
    xa_d = nc.declare_dram_parameter("xa", [128, s1, BATCH], bf16, isOutput=False)
    xb_d = nc.declare_dram_parameter("xb", [128, s2 - s1, BATCH], bf16, isOutput=False)
    xc_d = nc.declare_dram_parameter(
        "xc", [128, K_TILES - s2, BATCH], bf16, isOutput=False
    )
    out_d = nc.declare_dram_parameter("out", [O_SHARD, BATCH], bf16, isOutput=True)

    with contextlib.ExitStack() as stack:
        w_s = stack.enter_context(nc.sbuf_tensor([128, K_TILES, O_SHARD], bf16))
        x_s = stack.enter_context(nc.sbuf_tensor([128, K_TILES, BATCH], bf16))
        o_s = stack.enter_context(nc.sbuf_tensor([O_SHARD, BATCH], bf16))
        warm_s = stack.enter_context(nc.sbuf_tensor([128, 512], bf16))
        pt = stack.enter_context(nc.psum_tensor([O_SHARD, BATCH], f32))
        warm_pt = stack.enter_context(nc.psum_tensor([128, 512], f32))
        s_sp = stack.enter_context(nc.semaphore("s_sp"))
        s_act = stack.enter_context(nc.semaphore("s_act"))
        s_mm = stack.enter_context(nc.semaphore("s_mm"))
        s_cp = stack.enter_context(nc.semaphore("s_cp"))

        # Input streams: desc-gen at the head of both DMA-capable engines.
        nc.sync.dma_start(out=w_s[:], in_=w_d[:]).then_inc(s_sp, 16)
        nc.scalar.dma_start(out=x_s[:, 0:s1, :], in_=xa_d[:]).then_inc(s_act, 16)
        nc.scalar.dma_start(out=x_s[:, s1:s2, :], in_=xb_d[:]).then_inc(s_act, 16)
        nc.sync.dma_start(out=x_s[:, s2:, :], in_=xc_d[:]).then_inc(s_sp, 16)

        # PE warm-up on garbage SBUF into a scratch PSUM bank: builds the
        # sustained-busy window that releases the HAM clock gate while the
        # input DMAs stream.
        for _ in range(n_warm_big):
            nc.tensor.matmul(
                warm_pt[:], warm_s[:, 0:128], warm_s[:], start=True, stop=True
            )
        for _ in range(n_warm_small):
            nc.tensor.matmul(
                warm_pt[:, 0:64], warm_s[:, 0:128], warm_s[:, 0:64],
                start=True, stop=True,
            )

        # Real matmuls chase the x chunks.
        nc.tensor.wait_ge(s_sp, 16)  # w
        nc.tensor.wait_ge(s_act, 16)  # xa
        for k in range(K_TILES):
            if k == s1:
                nc.tensor.wait_ge(s_act, 32)  # xb
            if k == s2:
                nc.tensor.wait_ge(s_sp, 32)  # xc
            mm = nc.tensor.matmul(
                pt[:],
                w_s[:, k, :],
                x_s[:, k, :],
                start=(k == 0),
                stop=(k == K_TILES - 1),
            )
            if k == K_TILES - 1:
                mm.then_inc(s_mm, 1)

        # PSUM -> SBUF with fp32 -> bf16 cast, then the single output DMA.
        nc.vector.wait_ge(s_mm, 1)
        nc.vector.tensor_copy(out=o_s[:], in_=pt[:]).then_inc(s_cp, 1)
        nc.sync.wait_ge(s_cp, 1)
        nc.sync.dma_start(out=out_d[:], in_=o_s[:]).then_inc(s_sp, 16)

        # Epilogue: wait for the output receipt, then zero our sems so the
        # NEFF can re-execute (sems must be 0 at first DMA use).
        nc.sync.wait_ge(s_sp, 48)
        nums = sorted(s.num for s in (s_sp, s_act, s_mm, s_cp))
        assert nums == list(range(nums[0], nums[0] + 4)), nums
        nc.sync.sem_clear(range(nums[0], nums[-1] + 1))

        nc.compile()
    return nc


def _install_ntff_hook_shim():
    """The agent image's antenv lacks axon_hooks; recreate it so
    run_bass_kernel_spmd(trace=True) can capture NTFF profiles."""
    import sys
    import types

    if "antenv.axon_hooks" in sys.modules:
        return
    try:
        import antenv.axon_hooks  # noqa: F401  (real module exists)

        return
    except ImportError:
        pass
    mod = types.ModuleType("antenv.axon_hooks")
    mod._HOOK = None

    def set_axon_ntff_profile_hook(hook):
        mod._HOOK = hook

    def get_axon_ntff_profile_hook():
        return mod._HOOK

    mod.set_axon_ntff_profile_hook = set_axon_ntff_profile_hook
    mod.get_axon_ntff_profile_hook = get_axon_ntff_profile_hook
    sys.modules["antenv.axon_hooks"] = mod
    try:
        from trn_agent_boot.trn_boot import _ntff_profile_via_ctypes

        mod._HOOK = _ntff_profile_via_ctypes("/opt/axon/libaxon_pjrt.so")
    except Exception:
        pass


def _pack(a_t: np.ndarray, ncols: int) -> np.ndarray:
    """[SIZE_IN, ncols] f32 -> bf16 packed as [128, K_TILES, ncols]."""
    import ml_dtypes

    return np.ascontiguousarray(
        a_t.astype(ml_dtypes.bfloat16).reshape(K_TILES, 128, ncols).transpose(1, 0, 2)
    )


def kernel(x: np.ndarray, weights: np.ndarray, bias: np.ndarray) -> np.ndarray:
    from concourse.bass_utils import run_bass_kernel_spmd

    if "nc" not in _STATE:
        _STATE["nc"] = _build()
    nc = _STATE["nc"]

    s1 = int(os.environ.get("XSPLIT1", "3"))
    s2 = int(os.environ.get("XSPLIT2", "6"))

    x = np.asarray(x, dtype=np.float32)
    weights = np.asarray(weights, dtype=np.float32)
    bias = np.asarray(bias, dtype=np.float32)

    xt = _pack(np.ascontiguousarray(x.T), BATCH)  # [128, K_TILES, BATCH] bf16
    xa = np.ascontiguousarray(xt[:, 0:s1, :])
    xb = np.ascontiguousarray(xt[:, s1:s2, :])
    xc = np.ascontiguousarray(xt[:, s2:, :])
    wt = np.ascontiguousarray(weights.T)  # [SIZE_IN, SIZE_OUT] f32

    in_maps = []
    for c in range(N_CORES):
        sl = slice(c * O_SHARD, (c + 1) * O_SHARD)
        in_maps.append(
            {
                "w": _pack(np.ascontiguousarray(wt[:, sl]), O_SHARD),
                "xa": xa,
                "xb": xb,
                "xc": xc,
            }
        )

    # Always install the shim: if BASS_TRACE is set in the environment,
    # run_bass_kernel_spmd imports antenv.axon_hooks unconditionally and
    # would otherwise crash on images whose antenv lacks that module.
    _install_ntff_hook_shim()
    trace = os.environ.get("BASS_PROBLEM_TRACE", "0") == "1"
    res = run_bass_kernel_spmd(
        nc, in_maps, core_ids=list(range(N_CORES)), trace=trace
    )
    _STATE["last_results"] = res

    out_t = np.concatenate(
        [np.asarray(res.results[c]["out"]) for c in range(N_CORES)], axis=0
    )  # [SIZE_OUT, BATCH] bf16
    out = out_t.T.astype(np.float32) + bias[None, :]
    return np.ascontiguousarray(out)
